# revision 2
# baseline (speedup 1.0000x reference)
"""Trainium2 Bass kernel for nn_Decoder_recon (4-layer weight-shared transformer
decoder with agent-aware dual attention). Data-parallel: 8 samples -> 8 cores.

v2: fp8e4 DoubleRow matmuls for every K>=256 contraction (projections, FFN,
attention PV, output projections); scores stay fp8 plain (K=64). Weights are
pre-scaled x32 into fp8; the x1024 scale carried in residual PSUMs is absorbed
by the scale-invariance of LayerNorm.

Self-contained: hardcodes all shapes; only external dep is the Bass toolchain
at /opt/trn_rl_repo.
"""

import sys

sys.path.insert(0, "/opt/trn_rl_repo")

import numpy as np
import ml_dtypes

import concourse.bass as bass
import concourse.tile as tile
from concourse import mybir
from concourse.masks import make_identity

F32 = mybir.dt.float32
BF16 = mybir.dt.bfloat16
FP8 = mybir.dt.float8e4
NPBF16 = ml_dtypes.bfloat16
NPFP8 = ml_dtypes.float8_e4m3
AF = mybir.ActivationFunctionType
ALU = mybir.AluOpType
DR = mybir.MatmulPerfMode.DoubleRow

E, H, HD, DFF = 512, 8, 64, 2048
L, LK, S, NA, LF = 384, 256, 8, 32, 12
NL = 4
P = 128
NQ, NKV_SA, NKV_CA, NF, NFF = 3, 3, 2, 4, 16
EPS = 1e-5
SW = 32.0          # weight scale baked into fp8 weights
C = SW * SW        # psum scale on residual paths (1024)

# ---------------------------------------------------------------------------
# host-side prep (all SBUF-destined arrays are partition-first: [128, n, w])
# ---------------------------------------------------------------------------


def _pe_table(d_model=E, max_len=200):
    pos = np.arange(max_len, dtype=np.float32)[:, None]
    div = np.exp(
        np.arange(0, d_model, 2, dtype=np.float32) * (-np.log(10000.0) / d_model)
    )
    pe = np.zeros((max_len, d_model), dtype=np.float32)
    pe[:, 0::2] = np.sin(pos * div)
    pe[:, 1::2] = np.cos(pos * div)
    return pe


def _pfirst(a, n, w):
    """[n*128, w] -> [128, n, w] partition-first."""
    return np.ascontiguousarray(
        np.asarray(a, np.float32).reshape(n, P, w).transpose(1, 0, 2)
    )


def _fp8(a):
    return np.clip(np.asarray(a, np.float32), -240.0, 240.0).astype(NPFP8)


def _wt_fp8(w):
    """[out, in] weight -> lhsT layout [128, in/128, out], fp8 scaled x SW."""
    wt = np.ascontiguousarray(np.asarray(w, np.float32).T) * SW
    n_in = wt.shape[0]
    assert n_in % P == 0, n_in
    return _fp8(_pfirst(wt, n_in // P, wt.shape[1]))


def _bias_fm(b, scale=1.0):
    b = np.asarray(b, np.float32) * scale
    return _pfirst(b.reshape(-1, 1), b.size // P, 1).astype(np.float32)


def prep(inp):
    """Returns (shared dict name->array, per_core list of dicts)."""
    f32 = lambda x: np.asarray(x, np.float32)
    scale = 1.0 / np.sqrt(HD)
    v = f32(inp["v"])
    z = f32(inp["z"])
    v_enc = f32(inp["v_enc"])

    g = {}
    # folded input embedding: tgt0 = X0 @ wcomb.T + c0
    W1 = f32(inp["pos_fc_w"])[:, :E]
    W2 = f32(inp["pos_fc_w"])[:, E:]
    wcomb = W1 @ f32(inp["input_fc_w"])  # [512, 34]
    pos = np.repeat(_pe_table()[:LF], NA, axis=0)
    c0 = f32(inp["input_fc_b"]) @ W1.T + pos @ W2.T + f32(inp["pos_fc_b"])
    g["c0"] = _pfirst(c0, NQ, E).astype(NPBF16)  # [128, 3, 512] bf16
    wct = np.zeros((P, E), np.float32)
    wct[:34] = wcomb.T
    g["wcombt"] = wct.astype(NPBF16)

    for pfx in ("sa", "ca"):
        ipw, ipb = f32(inp[f"{pfx}_ipw"]), f32(inp[f"{pfx}_ipb"])
        ipw_s, ipb_s = f32(inp[f"{pfx}_ipw_s"]), f32(inp[f"{pfx}_ipb_s"])
        opw, opb = f32(inp[f"{pfx}_opw"]), f32(inp[f"{pfx}_opb"])
        g[f"{pfx}q_wt"] = _wt_fp8(ipw[:E] * scale)
        g[f"{pfx}q_b"] = _bias_fm(ipb[:E] * scale, SW)
        g[f"{pfx}k_wt"] = _wt_fp8(ipw[E : 2 * E])
        g[f"{pfx}k_b"] = _bias_fm(ipb[E : 2 * E], SW)
        g[f"{pfx}v_wt"] = _wt_fp8(ipw[2 * E :])
        g[f"{pfx}qs_wt"] = _wt_fp8(ipw_s[:E] * scale)
        g[f"{pfx}qs_b"] = _bias_fm(ipb_s[:E] * scale, SW)
        g[f"{pfx}ks_wt"] = _wt_fp8(ipw_s[E:])
        g[f"{pfx}ks_b"] = _bias_fm(ipb_s[E:], SW)
        g[f"{pfx}op_wt"] = _wt_fp8(opw)
        # v-bias folds into output-proj bias (softmax rows sum to 1)
        g[f"{pfx}op_brow"] = (
            ((opb + ipb[2 * E :] @ opw.T) * C).reshape(1, E).astype(NPBF16)
        )

    g["lin1_wt"] = _wt_fp8(f32(inp["lin1_w"]))
    g["lin1_b"] = _bias_fm(inp["lin1_b"], SW)
    g["lin2_wt"] = _wt_fp8(f32(inp["lin2_w"]))
    g["lin2_brow"] = (f32(inp["lin2_b"]) * C).reshape(1, E).astype(NPBF16)
    g["mlp1_wt"] = _wt_fp8(f32(inp["mlp1_w"]))
    g["mlp1_b"] = _bias_fm(inp["mlp1_b"], SW)
    g["mlp2_wt"] = _wt_fp8(f32(inp["mlp2_w"]))
    g["mlp2_b"] = _bias_fm(inp["mlp2_b"], SW)
    g["outfc_wt"] = _pfirst(f32(inp["out_fc_w"]).T / SW, 2, 2).astype(NPBF16)

    for nm in ("n1", "n2", "n3"):
        gg, bb = f32(inp[f"{nm}_g"]), f32(inp[f"{nm}_b"])
        g[f"{nm}_g"] = np.broadcast_to(gg, (P, E)).astype(np.float32).copy()
        g[f"{nm}_b"] = np.broadcast_to(bb, (P, E)).astype(np.float32).copy()
        g[f"{nm}_trivial"] = bool(np.all(gg == 1.0) and np.all(bb == 0.0))

    venct = np.ascontiguousarray(v_enc[:, 0, :].T)  # [512, 256]
    g["venct"] = _fp8(_pfirst(venct, NF, LK))

    pp = np.arange(P)[:, None] % NA
    cc = np.arange(L)[None, :] % NA
    g["mself"] = (pp == cc).astype(np.uint8)
    # within a 128-token diagonal block: key row r (time r//32) may not see
    # query col c (time c//32) when r//32 > c//32
    rr = np.arange(P)[:, None] // NA
    tt = np.arange(P)[None, :] // NA
    g["trimask"] = (rr <= tt).astype(np.float32).astype(NPFP8)

    F = (
        f32(inp["out_fc_b"])[None, :]
        + np.tile(v[0, 0], (LF, 1))
        + f32(inp["scene_norm"])[None, :]
    )
    g["fadd"] = _pfirst(F, NQ, 2).astype(np.float32)

    dec_flat = v[0].reshape(L, 2)
    z3 = z.reshape(L, S, -1)
    per_core = []
    for s in range(S):
        x0 = np.concatenate([dec_flat, z3[:, s, :]], axis=-1)  # [384, 34]
        x0t = np.zeros((P, L), np.float32)
        x0t[:34] = x0.T
        per_core.append({"x0t": x0t.astype(NPBF16)})
    return g, per_core


_BIAS_NAMES = ("saq_b", "sak_b", "saqs_b", "saks_b", "caq_b", "cak_b",
               "caqs_b", "caks_b", "lin1_b", "mlp1_b", "mlp2_b")
_ROW_NAMES = ("saop_brow", "caop_brow", "lin2_brow")


def _flags(g):
    bias_nz = tuple((nm, bool(np.any(np.asarray(g[nm]) != 0))) for nm in _BIAS_NAMES)
    row_nz = tuple(
        (nm, bool(np.any(np.asarray(g[nm], np.float32) != 0))) for nm in _ROW_NAMES
    )
    ln_triv = tuple(g[f"{nm}_trivial"] for nm in ("n1", "n2", "n3"))
    return (bias_nz, row_nz, ln_triv)


# ---------------------------------------------------------------------------
# device kernel
# ---------------------------------------------------------------------------

_WEIGHT_SPECS = [
    ("wcombt", (P, E), BF16),
    ("venct", (P, NF, LK), FP8),
    ("mself", (P, L), mybir.dt.uint8),
    ("trimask", (P, P), FP8),
    ("fadd", (P, NQ, 2), F32),
    ("lin1_wt", (P, NF, DFF), FP8),
    ("lin2_wt", (P, NFF, E), FP8),
    ("mlp1_wt", (P, NF, E), FP8),
    ("mlp2_wt", (P, NF, 256), FP8),
    ("outfc_wt", (P, 2, 2), BF16),
] + [
    (f"{pfx}{nm}_wt", (P, NF, E), FP8)
    for pfx in ("sa", "ca")
    for nm in ("q", "k", "v", "qs", "ks", "op")
]

_BIAS_N = {"lin1_b": NFF, "mlp2_b": 2}
DBG = False


def _split_multi_waits(nc):
    """Walrus codegen allows one sync-wait per instruction; hoist extras onto
    engine-local InstNoOps inserted just before the offending instruction."""
    n_split = 0
    for fn in nc.m.functions:
        for bb in fn.blocks:
            il = bb.instructions
            i = 0
            while i < len(il):
                inst = il[i]
                si = inst.sync_info
                if si is not None and si.on_wait and len(si.on_wait) > 1:
                    waits = list(si.on_wait)
                    for w in waits[:-1]:
                        nop = mybir.InstNoOp(
                            name=nc.get_next_instruction_name(),
                            sync_info=mybir.SyncInfo(on_wait=[w], on_update=[]),
                            engine=inst.engine,
                            bass_nofuse=True,
                        )
                        nc.register_instruction(nop, overwrite=True)
                        il.insert(i, nop)
                        i += 1
                        n_split += 1
                    inst.sync_info = mybir.SyncInfo(
                        on_wait=[waits[-1]], on_update=list(si.on_update)
                    )
                i += 1
    return n_split


def build(flags):
    bias_nz = dict(flags[0])
    row_nz = dict(flags[1])
    ln_triv = flags[2]

    nc = bass.Bass()
    dram = {}
    # DMA issue order follows this declaration order: embed inputs + SA weights
    # first so compute starts while CA/FFN/head weights stream in.
    order = ["x0t_decl", "wcombt", "c0_decl", "mself", "trimask",
             "sak_wt", "saks_wt", "sav_wt", "saq_wt", "saqs_wt", "saop_wt",
             "venct", "caq_wt", "cak_wt", "caqs_wt", "caks_wt", "cav_wt",
             "caop_wt", "lin1_wt", "lin2_wt", "mlp1_wt", "mlp2_wt",
             "outfc_wt", "fadd"]
    spec_by_name = {nm: (shp, dt) for nm, shp, dt in _WEIGHT_SPECS}
    for nm, shp, dt in _WEIGHT_SPECS:
        dram[nm] = nc.declare_dram_parameter(nm, list(shp), dt, isOutput=False)
    dram["c0"] = nc.declare_dram_parameter("c0", [P, NQ, E], BF16, isOutput=False)
    extra_f32 = []
    for nm, on in bias_nz.items():
        if on:
            extra_f32.append((nm, [P, _BIAS_N.get(nm, NF), 1]))
    for nm, on in row_nz.items():
        if on:
            dram[nm] = nc.declare_dram_parameter(nm, [1, E], BF16, isOutput=False)
    for i, triv in enumerate(ln_triv):
        if not triv:
            extra_f32.append((f"n{i+1}_g", [P, E]))
            extra_f32.append((f"n{i+1}_b", [P, E]))
    for nm, shp in extra_f32:
        dram[nm] = nc.declare_dram_parameter(nm, shp, F32, isOutput=False)
    dram["x0t"] = nc.declare_dram_parameter("x0t", [P, L], BF16, isOutput=False)
    out_dram = nc.declare_dram_parameter("out", [P, NQ, 2], F32, isOutput=True)
    dbg_dram = None
    if DBG:
        dbg_dram = nc.declare_dram_parameter("dbg", [P, 16, NQ, E], F32,
                                             isOutput=True)
    dbg_idx = [0]

    with tile.TileContext(nc) as tc, \
         tc.tile_pool(name="singles", bufs=1) as singles, \
         tc.tile_pool(name="work", bufs=2) as sb, \
         tc.tile_pool(name="expp", bufs=2) as sb3, \
         tc.tile_pool(name="small", bufs=6) as small, \
         tc.tile_pool(name="ps_sc", bufs=2, space="PSUM") as ps_sc, \
         tc.tile_pool(name="ps_mm", bufs=3, space="PSUM") as ps_mm, \
         tc.tile_pool(name="ps_pv", bufs=1, space="PSUM") as ps_pv:

        # ---- load inputs (ordered for early compute start)
        W = {}
        x0t = None
        c0_sb = None
        for nm in order:
            if nm == "x0t_decl":
                x0t = singles.tile([P, L], BF16, tag="x0t", name="x0t")
                nc.sync.dma_start(out=x0t, in_=dram["x0t"][:])
            elif nm == "c0_decl":
                c0_sb = singles.tile([P, NQ, E], BF16, tag="c0", name="c0")
                nc.sync.dma_start(out=c0_sb, in_=dram["c0"][:])
            else:
                shp, dt = spec_by_name[nm]
                W[nm] = singles.tile(list(shp), dt, tag=nm, name=nm)
                nc.sync.dma_start(out=W[nm], in_=dram[nm][:])
        for nm, on in row_nz.items():
            if on:
                W[nm] = singles.tile([1, E], BF16, tag=nm, name=nm)
                nc.sync.dma_start(out=W[nm], in_=dram[nm][:])
        for nm, shp in extra_f32:
            W[nm] = singles.tile(shp, F32, tag=nm, name=nm)
            nc.sync.dma_start(out=W[nm], in_=dram[nm][:])

        ident_bf16 = singles.tile([P, P], BF16, tag="idb", name="idb")
        make_identity(nc, ident_bf16)
        # residual identity carrying the x1024 psum scale of fp8 paths
        ident_hi = singles.tile([P, P], BF16, tag="idh", name="idh")
        make_identity(nc, ident_hi)
        nc.vector.tensor_scalar_mul(out=ident_hi, in0=ident_hi, scalar1=C)
        eps_t = singles.tile([P, 1], F32, tag="eps", name="eps")
        nc.vector.memset(eps_t, EPS * C * C)
        ones_row = singles.tile([1, P], BF16, tag="ones", name="ones")
        nc.vector.memset(ones_row, 1.0)

        mself = W["mself"]
        trimask = W["trimask"]
        # residual stream: three independent bf16 tiles (per token block)
        tgt = [singles.tile([P, E], BF16, tag=f"tgt{i}", name=f"tgt{i}")
               for i in range(NQ)]
        # v_aug tiles [P, nkv, H, 65] fp8 (ones column initialized once)
        va_sa = singles.tile([P, NKV_SA, H, 65], FP8, tag="vasa", name="vasa")
        va_ca = singles.tile([P, NKV_CA, H, 65], FP8, tag="vaca", name="vaca")
        for t in (va_sa, va_ca):
            nc.gpsimd.memset(t[:, :, :, 64:65], 1.0)

        def bias_ap(nm, fo):
            if nm is not None and bias_nz.get(nm, False):
                return W[nm][:, fo, :]
            return 0.0

        def proj_fm(x_fm, wt, n_out, b_nm, tag, relu=False, n_in=NF, width=L,
                    out_dt=FP8, pool=sb, bufs=None):
            """[P, n_out, width] tile: rows of (W @ X.T), fp8 by default.
            x_fm is a [P, n_in, width] fp8 tile; DoubleRow over ki pairs."""
            o = pool.tile([P, n_out, width], out_dt, tag=tag, name=tag,
                          bufs=bufs)
            for fo in range(n_out):
                pm = ps_mm.tile([P, width], F32, tag="mm", name="pm")
                for kp in range(n_in // 2):
                    nc.tensor.matmul(
                        pm,
                        wt[:, 2 * kp : 2 * kp + 2, fo * P : (fo + 1) * P],
                        x_fm[:, 2 * kp : 2 * kp + 2, :],
                        start=(kp == 0),
                        stop=(kp == n_in // 2 - 1),
                        perf_mode=DR,
                    )
                nc.scalar.activation(
                    out=o[:, fo, :], in_=pm, func=AF.Relu if relu else AF.Copy,
                    bias=bias_ap(b_nm, fo),
                )
            return o

        def transpose_to_fm(tag):
            """Transpose tgt -> feature-major fp8 tile [P, NF, L]."""
            x_fm = sb.tile([P, NF, L], FP8, tag=tag, name=tag)
            for f in range(NF):
                pt = ps_mm.tile([P, L], BF16, tag="mm", name="pt")
                for i in range(NQ):
                    nc.tensor.matmul(
                        pt[:, i * P : (i + 1) * P],
                        tgt[i][:, f * P : (f + 1) * P],
                        ident_bf16,
                        is_transpose=True,
                        start=(i == 0),
                        stop=(i == NQ - 1),
                    )
                nc.vector.tensor_copy(out=x_fm[:, f, :], in_=pt)
            return x_fm

        def fill_v_aug(x_fm, wt, va, nkv):
            pms = [ps_mm.tile([P, E], F32, tag="mm", name=f"vpm{t}")
                   for t in range(nkv)]
            for kp in range(NF // 2):
                for t in range(nkv):
                    nc.tensor.matmul(
                        pms[t],
                        x_fm[:, 2 * kp : 2 * kp + 2, t * P : (t + 1) * P],
                        wt[:, 2 * kp : 2 * kp + 2, :],
                        start=(kp == 0),
                        stop=(kp == NF // 2 - 1),
                        perf_mode=DR,
                    )
            for t in range(nkv):
                nc.scalar.activation(
                    out=va[:, t, :, 0:64],
                    in_=pms[t].rearrange("p (h d) -> p h d", d=64),
                    func=AF.Copy,
                )

        def attention(x_fm, q_wt, q_b, qs_wt, qs_b, k_fm, ks_fm, va,
                      nkv, causal, tp):
            q_fm = proj_fm(x_fm, q_wt, NF, q_b, tp + "q", bufs=1)
            qs_fm = proj_fm(x_fm, qs_wt, NF, qs_b, tp + "qs", bufs=1)
            o_tm = [sb.tile([P, NQ, P], BF16, tag=f"{tp}otm{f}",
                            name=f"otm{f}", bufs=1) for f in range(4)]
            o_fm = sb.tile([P, NF, L], FP8, tag=f"{tp}ofm", name=f"{tp}ofm",
                           bufs=1)
            pv2 = [None]

            def scores_blend_exp(h):
                fpair, koff = h // 2, (h % 2) * 64
                ex = sb3.tile([P, nkv, L], FP8, tag=f"{tp}ex", name="ex")
                for j in range(nkv):
                    qoff = P * j if causal else 0
                    wdt = L - qoff
                    psc = ps_sc.tile([P, 2, 512], F32, tag="sc", name="psc")
                    nc.tensor.matmul(
                        psc[:, 0, :wdt],
                        ks_fm[koff : koff + 64, fpair, j * P : (j + 1) * P],
                        qs_fm[koff : koff + 64, fpair, qoff:L],
                        start=True, stop=True,
                    )
                    nc.tensor.matmul(
                        psc[:, 1, :wdt],
                        k_fm[koff : koff + 64, fpair, j * P : (j + 1) * P],
                        q_fm[koff : koff + 64, fpair, qoff:L],
                        start=True, stop=True,
                    )
                    nc.vector.copy_predicated(
                        out=psc[:, 1, :wdt],
                        mask=mself[:, :wdt],
                        data=psc[:, 0, :wdt],
                    )
                    nc.scalar.activation(
                        out=ex[:, j, qoff:L], in_=psc[:, 1, :wdt],
                        func=AF.Exp, scale=1.0 / C,
                    )
                    if causal:
                        # zero the within-block lower triangle (tk > tq)
                        nc.gpsimd.tensor_mul(
                            out=ex[:, j, qoff : qoff + P],
                            in0=ex[:, j, qoff : qoff + P],
                            in1=trimask,
                        )
                return ex

            def pv_mm(h, ex):
                fpair, hl = h // 2, h % 2
                if hl == 0:
                    pv2[0] = ps_pv.tile([P, NQ, 130], F32, tag="pv", name="pv")
                pv = pv2[0]
                col = hl * 65
                for i in range(NQ):
                    njs = (i + 1) if causal else nkv
                    o_ap = pv[:, i, col : col + 65]
                    done = 0
                    while done < njs:
                        if njs - done >= 2:
                            nc.tensor.matmul(
                                o_ap,
                                ex[:, done : done + 2, i * P : (i + 1) * P],
                                va[:, done : done + 2, h, :],
                                start=(done == 0),
                                stop=(done + 2 == njs),
                                perf_mode=DR,
                            )
                            done += 2
                        else:
                            nc.tensor.matmul(
                                o_ap,
                                ex[:, done, i * P : (i + 1) * P],
                                va[:, done, h, :],
                                start=(done == 0),
                                stop=True,
                            )
                            done += 1
                if hl == 1:
                    pvv = pv.rearrange("p i (t s) -> p i t s", t=2)
                    rec2 = small.tile([P, NQ, 2, 1], F32, tag="rec", name="rec")
                    nc.vector.reciprocal(rec2, pvv[:, :, :, 64:65])
                    nc.vector.tensor_mul(
                        out=o_tm[fpair].rearrange("p i (t s) -> p i t s", t=2),
                        in0=pvv[:, :, :, 0:64],
                        in1=rec2.broadcast_to([P, NQ, 2, 64]),
                    )
                    ptr = ps_mm.tile([P, L], BF16, tag="mm", name="ptr")
                    for i in range(NQ):
                        nc.tensor.matmul(
                            ptr[:, i * P : (i + 1) * P],
                            o_tm[fpair][:, i, :],
                            ident_bf16,
                            is_transpose=True,
                            start=(i == 0),
                            stop=(i == NQ - 1),
                        )
                    nc.vector.tensor_copy(out=o_fm[:, fpair, :], in_=ptr)

            # software-pipelined: head h's PV trails head h+1's scores
            pend = None
            for h in range(H):
                ex = scores_blend_exp(h)
                if pend is not None:
                    pv_mm(*pend)
                pend = (h, ex)
            pv_mm(*pend)
            return o_fm

        def contract_to_tm(src_fm, wt, n_in, brow_nm):
            """Token-major psum tiles; DoubleRow over ki pairs; residual (tgt,
            x1024) and bias row fold into the same PE accumulation group."""
            add_row = row_nz.get(brow_nm, False)
            pms = [ps_mm.tile([P, E], F32, tag="mm", name=f"pm{i}")
                   for i in range(NQ)]
            for kp in range(n_in // 2):
                for i in range(NQ):
                    nc.tensor.matmul(
                        pms[i],
                        src_fm[:, 2 * kp : 2 * kp + 2, i * P : (i + 1) * P],
                        wt[:, 2 * kp : 2 * kp + 2, :],
                        start=(kp == 0),
                        stop=False,
                        perf_mode=DR,
                    )
            for i in range(NQ):
                if add_row:
                    nc.tensor.matmul(pms[i], ones_row, W[brow_nm], start=False,
                                     stop=False)
                # residual add on PE: pm += (C*I).T @ tgt
                nc.tensor.matmul(pms[i], ident_hi, tgt[i], start=False,
                                 stop=True)
            return pms

        def dbg_dump():
            if dbg_dram is not None:
                for i in range(NQ):
                    f32c = small.tile([P, E], F32, tag="dbgc", name="dbgc")
                    nc.vector.tensor_copy(out=f32c, in_=tgt[i])
                    nc.sync.dma_start(out=dbg_dram[:, dbg_idx[0], i, :], in_=f32c)
                dbg_idx[0] += 1

        def residual_ln(pms, ln_idx):
            triv = ln_triv[ln_idx]
            for i in range(NQ):
                stats = small.tile([P, 6], F32, tag="bnst", name="stats")
                nc.vector.bn_stats(stats, pms[i])
                mv = small.tile([P, 2], F32, tag="bnmv", name="mv")
                nc.vector.bn_aggr(mv, stats)
                std = small.tile([P, 1], F32, tag="std", name="std")
                nc.scalar.activation(out=std, in_=mv[:, 1:2], func=AF.Sqrt,
                                     bias=eps_t)
                rstd = small.tile([P, 1], F32, tag="rstd", name="rstd")
                nc.vector.reciprocal(rstd, std)
                nc.vector.tensor_scalar(
                    out=tgt[i], in0=pms[i],
                    scalar1=mv[:, 0:1], scalar2=rstd,
                    op0=ALU.subtract, op1=ALU.mult,
                )
                if not triv:
                    nc.vector.tensor_mul(out=tgt[i], in0=tgt[i],
                                         in1=W[f"n{ln_idx+1}_g"])
                    nc.vector.tensor_add(out=tgt[i], in0=tgt[i],
                                         in1=W[f"n{ln_idx+1}_b"])
            dbg_dump()

        # ---- input embedding: tgt = c0 + (X0 @ wcomb.T)
        for i in range(NQ):
            pm = ps_mm.tile([P, E], F32, tag="mm", name="pm")
            nc.tensor.matmul(
                pm, x0t[:, i * P : (i + 1) * P], W["wcombt"], start=True,
                stop=True,
            )
            nc.vector.tensor_add(out=tgt[i], in0=c0_sb[:, i, :], in1=pm)
        dbg_dump()

        # ---- cross-attn K/V/Ks (fixed across layers)
        venct = W["venct"]
        kc_fm = proj_fm(venct, W["cak_wt"], NF, "cak_b", "kc", width=LK,
                        pool=singles)
        ksc_fm = proj_fm(venct, W["caks_wt"], NF, "caks_b", "ksc", width=LK,
                         pool=singles)
        fill_v_aug(venct, W["cav_wt"], va_ca, NKV_CA)

        # ---- decoder layers (shared weights)
        for _layer in range(NL):
            x_fm = transpose_to_fm("x_fm")
            k_fm = proj_fm(x_fm, W["sak_wt"], NF, "sak_b", "k_fm", bufs=1)
            ks_fm = proj_fm(x_fm, W["saks_wt"], NF, "saks_b", "ks_fm", bufs=1)
            fill_v_aug(x_fm, W["sav_wt"], va_sa, NKV_SA)
            o_fm = attention(
                x_fm, W["saq_wt"], "saq_b", W["saqs_wt"], "saqs_b",
                k_fm, ks_fm, va_sa, NKV_SA, True, "sa",
            )
            residual_ln(contract_to_tm(o_fm, W["saop_wt"], NF, "saop_brow"), 0)

            x_fm = transpose_to_fm("x_fm")
            o_fm = attention(
                x_fm, W["caq_wt"], "caq_b", W["caqs_wt"], "caqs_b",
                kc_fm, ksc_fm, va_ca, NKV_CA, False, "ca",
            )
            residual_ln(contract_to_tm(o_fm, W["caop_wt"], NF, "caop_brow"), 1)

            x_fm = transpose_to_fm("x_fm")
            h_fm = proj_fm(x_fm, W["lin1_wt"], NFF, "lin1_b", "ff", relu=True,
                           bufs=1)
            residual_ln(contract_to_tm(h_fm, W["lin2_wt"], NFF, "lin2_brow"), 2)

        # ---- head MLP
        x_fm = transpose_to_fm("x_fm")
        h1 = proj_fm(x_fm, W["mlp1_wt"], NF, "mlp1_b", "m1", relu=True)
        # mlp2: psum = C * z2; store h2 = SW * relu(z2) in bf16
        h2 = sb.tile([P, 2, L], BF16, tag="m2", name="m2")
        for fo in range(2):
            pm = ps_mm.tile([P, L], F32, tag="mm", name="pm")
            for kp in range(NF // 2):
                nc.tensor.matmul(
                    pm,
                    W["mlp2_wt"][:, 2 * kp : 2 * kp + 2, fo * P : (fo + 1) * P],
                    h1[:, 2 * kp : 2 * kp + 2, :],
                    start=(kp == 0),
                    stop=(kp == NF // 2 - 1),
                    perf_mode=DR,
                )
            nc.scalar.activation(
                out=h2[:, fo, :], in_=pm, func=AF.Relu, scale=1.0 / SW,
                bias=bias_ap("mlp2_b", fo),
            )
        for i in range(NQ):
            pm = ps_mm.tile([P, 2], F32, tag="mm", name="pm")
            for ki in range(2):
                nc.tensor.matmul(
                    pm,
                    h2[:, ki, i * P : (i + 1) * P],
                    W["outfc_wt"][:, ki, :],
                    start=(ki == 0),
                    stop=(ki == 1),
                )
            o = small.tile([P, 2], F32, tag="outt", name="o")
            nc.vector.tensor_add(out=o, in0=W["fadd"][:, i, :], in1=pm)
            nc.sync.dma_start(out=out_dram[:, i, :], in_=o)

    _split_multi_waits(nc)
    return nc


# ---------------------------------------------------------------------------
# runner
# ---------------------------------------------------------------------------

_CACHE = {}


def _get_built(flags):
    if flags not in _CACHE:
        _CACHE[flags] = build(flags)
    return _CACHE[flags]


def make_in_maps(g, per_core):
    flags = _flags(g)
    bias_nz, row_nz, ln_triv = dict(flags[0]), dict(flags[1]), flags[2]
    shared = {nm: g[nm] for nm, _, _ in _WEIGHT_SPECS}
    shared["c0"] = g["c0"]
    for nm, on in bias_nz.items():
        if on:
            shared[nm] = g[nm]
    for nm, on in row_nz.items():
        if on:
            shared[nm] = g[nm]
    for i, triv in enumerate(ln_triv):
        if not triv:
            shared[f"n{i+1}_g"] = g[f"n{i+1}_g"]
            shared[f"n{i+1}_b"] = g[f"n{i+1}_b"]
    return flags, [{**shared, **pc} for pc in per_core]


def _postprocess(results):
    outs = []
    for s in range(S):
        o = np.asarray(results[s]["out"], np.float32)  # [128, 3, 2]
        o = o.transpose(1, 0, 2).reshape(L, 2)
        outs.append(o.reshape(LF, NA, 2))
    return np.stack(outs).astype(np.float32)


def run_on_hw(g, per_core, trace=False, **kw):
    from concourse.bass_utils import run_bass_kernel_spmd

    flags, in_maps = make_in_maps(g, per_core)
    nc = _get_built(flags)
    return run_bass_kernel_spmd(nc, in_maps, list(range(S)), trace=trace, **kw)


def kernel(**inputs):
    g, per_core = prep(inputs)
    res = run_on_hw(g, per_core)
    return _postprocess(res.results)
